# revision 1
# baseline (speedup 1.0000x reference)
"""Variable-length average pooling (prefix mean over seq axis) on 8 trn2 cores.

Strategy (pure data parallelism over batch, host-side repack to fp16):
  - eff_len[b] = lengths[b] if >0 else L.  pooled[b] = sum_{l<eff} x[b,l,:] / eff.
  - Memory-regime problem: the only bytes the device must touch are the valid
    rows. Host-side levers cut HBM traffic to the floor:
      1. Pack ONLY the valid prefix rows of each batch per core (sorted+snake
         assignment of 16 batches/core balances totals to ~0.8%).
      2. Ship rows as fp16 (features are N(0,1); norm rel err ~3.7e-4, ~50x
         inside the 2e-2 gate) - halves HBM bytes.
    Net ~35.7 MB/core; measured HWDGE streaming rate is ~410 GB/s/core, so
    the DMA floor is ~87 us (fp32 batch-aligned baseline: 269 us).
  - TWIN-PAIR packing keeps both compute engines far below the DMA floor:
    cells (j, p) hold TWO rows of the SAME batch - row p of chunk 2j and row
    p of chunk 2j+1 share slot and mask weight. Odd last rows are DUPLICATED
    into both halves of a cell with halved weight (x+x)*(1/2e) = x/e, so
    every cell is same-slot by construction. The DVE pre-adds each pair
    (fp16, ~40 us total) and the PE mask-matmuls the SUM:
        psum[16, 512q] += mask_j[128, 16].T @ (tileA + tileB)[128, 512q]
    (~40 us PE busy at full fp16 rate). A single PSUM accumulation group per
    512-col bank spans all pair-blocks. mask[p, 16j+s] = cell weight if cell
    (j, p) belongs to slot s else 0, host-built [128, NPB*16] fp16.
  - DMA layout: the host writes each DMA group's bytes PARTITION-MAJOR, so
    every partition's load is one contiguous 16 KiB descriptor (4x fewer,
    4x larger descriptors than row-major chunk layout). Groups of 2
    pair-blocks (2 MiB) alternate the two HWDGE rings (SP/ACT); the first
    two groups are single blocks so the first matmul isn't stuck behind
    megabytes of prefetch on the shared SDMA engines. The mask rides the
    GpSimd SWDGE queue, off both HWDGE rings.
  - Tail: PSUM->SBUF copy on DVE (ACT reading PSUM crashes the exec unit;
    DVE is the sanctioned path), then one DMA out per core; host scatters
    rows back to batch order.
"""

import os

import numpy as np

import concourse.bacc as bacc
import concourse.mybir as mybir
from concourse.tile import TileContext
from concourse.bass_utils import run_bass_kernel_spmd

B, L, D = 128, 1024, 2048
NCORES = 8
SLOTS = B // NCORES  # 16
P = 128              # cells per pair-block (partition dim)
NTILE = 512          # matmul moving free dim (one PSUM bank of fp32)

GROUP = int(os.environ.get("DMA_GROUP", "2"))        # pair-blocks per DMA
FIRST_SINGLES = int(os.environ.get("FIRST_SINGLES", "0"))
TILE_BUFS = int(os.environ.get("TILE_BUFS", "6"))
SUM_BUFS = int(os.environ.get("SUM_BUFS", "8"))

LAST_RESULTS = None  # BassKernelResults of the most recent device run


def _plan(eff):
    """Snake-assign sorted batches to cores; return (cores, npairblocks)."""
    order = np.argsort(-eff, kind="stable")
    cores = [[] for _ in range(NCORES)]
    for i, idx in enumerate(order):
        blk, pos = divmod(i, NCORES)
        c = pos if blk % 2 == 0 else NCORES - 1 - pos
        cores[c].append(int(idx))
    max_cells = max(
        sum((int(eff[b]) + 1) // 2 for b in perm) for perm in cores
    )
    npb = -(-max_cells // P)
    return cores, npb


def _groups(npb):
    """DMA group sizes: FIRST_SINGLES single blocks, GROUP-block runs, then
    single blocks for the final <=GROUP blocks (shortens the add+matmul trail
    that runs after the last HBM byte lands)."""
    out, j = [], 0
    while j < npb:
        if len(out) < FIRST_SINGLES or npb - j <= GROUP:
            gl = 1
        else:
            gl = min(GROUP, npb - j)
        out.append((j, gl))
        j += gl
    return out


_PROGRAM_CACHE = {}


def _build_program(npb):
    # Bacc (not raw Bass): its compile pass splits multi-sem waits and moves
    # matmul waits onto ldweights — walrus allows only 1 wait per instruction.
    nc = bacc.Bacc(None, target_bir_lowering=False)
    f16 = mybir.dt.float16
    f32 = mybir.dt.float32
    packed = nc.dram_tensor("packed", [npb * 2 * P * D], f16, kind="ExternalInput")
    maskt = nc.dram_tensor("maskt", [P, npb * SLOTS], f16, kind="ExternalInput")
    out = nc.dram_tensor("out", [SLOTS, D], f32, kind="ExternalOutput")

    with TileContext(nc) as tc:
        with (
            tc.tile_pool(name="mask", bufs=1) as mpool,
            tc.tile_pool(name="tiles", bufs=TILE_BUFS) as tpool,
            tc.tile_pool(name="sums", bufs=SUM_BUFS) as spool,
            tc.tile_pool(name="psum", bufs=1, space="PSUM") as ppool,
            tc.tile_pool(name="outs", bufs=1) as opool,
        ):
            mask_tile = mpool.tile([P, npb * SLOTS], f16)
            # One PSUM tile per 512-col bank so each bank's tail copy only
            # depends on ITS accumulation group's stop, not the whole psum.
            psums = [
                ppool.tile([SLOTS, NTILE], f32, name=f"ps{q}", tag=f"ps{q}")
                for q in range(D // NTILE)
            ]

            # Mask via SWDGE (GpSimd) so both HWDGE rings lead with data.
            nc.gpsimd.dma_start(out=mask_tile[:], in_=maskt[:])
            dma_engines = [nc.sync, nc.scalar]
            for n_dma, (j0, gl) in enumerate(_groups(npb)):
                tile = tpool.tile([P, gl * 2 * D], f16, name=f"t{gl}", tag="t")
                off = j0 * 2 * P * D
                cnt = gl * 2 * P * D
                src = packed[off : off + cnt].rearrange("(p x) -> p x", p=P)
                dma_engines[n_dma % 2].dma_start(out=tile[:], in_=src)
                for g in range(gl):
                    jj = j0 + g
                    c = 2 * g
                    sum2 = spool.tile([P, D], f16, name="sum2", tag="s")
                    nc.vector.tensor_add(
                        out=sum2[:],
                        in0=tile[:, c * D : (c + 1) * D],
                        in1=tile[:, (c + 1) * D : (c + 2) * D],
                    )
                    for q in range(D // NTILE):
                        nc.tensor.matmul(
                            psums[q][:, :],
                            mask_tile[:, jj * SLOTS : (jj + 1) * SLOTS],
                            sum2[:, q * NTILE : (q + 1) * NTILE],
                            start=(jj == 0),
                            stop=(jj == npb - 1),
                        )

            # Tail: PSUM->SBUF via DVE (ACT reading PSUM crashes the exec
            # unit on this part). Bank q's copy chases its own group stop
            # (the last block's MM q), and each piece's out-DMA dispatch
            # hides under the next piece's copy.
            out_t = opool.tile([SLOTS, D], f32)
            for q in range(D // NTILE):
                nc.vector.tensor_copy(
                    out=out_t[:, q * NTILE : (q + 1) * NTILE], in_=psums[q][:, :]
                )
                dma_engines[q % 2].dma_start(
                    out=out[:, q * NTILE : (q + 1) * NTILE],
                    in_=out_t[:, q * NTILE : (q + 1) * NTILE],
                )
    nc.finalize()
    return nc


def kernel(features, lengths):
    global LAST_RESULTS
    features = np.ascontiguousarray(features, dtype=np.float32)
    lengths = np.ascontiguousarray(lengths, dtype=np.int32)
    eff = np.where(lengths > 0, lengths, L).astype(np.int64)

    cores, npb = _plan(eff)
    key = (npb, GROUP, FIRST_SINGLES, TILE_BUFS, SUM_BUFS)
    if key not in _PROGRAM_CACHE:
        _PROGRAM_CACHE[key] = _build_program(npb)
    nc = _PROGRAM_CACHE[key]
    groups = _groups(npb)

    f16rows = features.astype(np.float16).reshape(B * L, D)
    in_maps = []
    for c in range(NCORES):
        perm = cores[c]
        ncell = npb * P
        idxA = np.zeros(ncell, dtype=np.int64)
        idxB = np.zeros(ncell, dtype=np.int64)
        wts = np.zeros(ncell, dtype=np.float32)
        slot = np.zeros(ncell, dtype=np.int64)
        o = 0
        for s, b in enumerate(perm):
            e = int(eff[b])
            base = b * L
            npairs = e // 2
            ar = np.arange(npairs, dtype=np.int64)
            idxA[o : o + npairs] = base + 2 * ar
            idxB[o : o + npairs] = base + 2 * ar + 1
            wts[o : o + npairs] = 1.0 / e
            slot[o : o + npairs] = s
            o += npairs
            if e % 2:
                idxA[o] = idxB[o] = base + e - 1
                wts[o] = 0.5 / e
                slot[o] = s
                o += 1
        # padding cells keep idx 0 (finite garbage) with weight 0
        A3 = f16rows[idxA].reshape(npb, P, D)
        B3 = f16rows[idxB].reshape(npb, P, D)
        flat = np.empty(npb * 2 * P * D, dtype=np.float16)
        for j0, gl in groups:
            off = j0 * 2 * P * D
            cnt = gl * 2 * P * D
            seg = flat[off : off + cnt].reshape(P, 2 * gl, D)
            seg[:, 0::2] = A3[j0 : j0 + gl].transpose(1, 0, 2)
            seg[:, 1::2] = B3[j0 : j0 + gl].transpose(1, 0, 2)
        maskflat = np.zeros((ncell, SLOTS), dtype=np.float32)
        maskflat[np.arange(ncell), slot] = wts
        maskt = np.ascontiguousarray(
            maskflat.astype(np.float16)
            .reshape(npb, P, SLOTS)
            .transpose(1, 0, 2)
            .reshape(P, npb * SLOTS)
        )
        in_maps.append({"packed": flat, "maskt": maskt})

    trace = os.environ.get("KERNEL_TRACE", "0") == "1"
    LAST_RESULTS = run_bass_kernel_spmd(
        nc,
        in_maps,
        core_ids=list(range(NCORES)),
        trace=trace,
        trace_cores=[0] if trace else None,
    )

    out = np.empty((B, D), dtype=np.float32)
    for c in range(NCORES):
        out[np.asarray(cores[c])] = LAST_RESULTS.results[c]["out"]
    return out



# revision 2
# speedup vs baseline: 1.9331x; 1.9331x over previous
"""Variable-length average pooling (prefix mean over seq axis) on 8 trn2 cores.

Strategy (data parallel over batch; host repack to fp8 + DoubleRow mask-matmul):
  - eff_len[b] = lengths[b] if >0 else L.  pooled[b] = sum_{l<eff} x[b,l,:] / eff.
  - Memory-regime: only the valid prefix rows must reach the device. Two host
    levers cut HBM traffic to ~17.4 MB/core (fp32 batch-aligned: 128 MB):
      1. Pack ONLY valid prefix rows (sorted+snake assignment of 16
         batches/core balances totals to ~1%).
      2. Ship rows as fp8e4m3 with ERROR-FEEDBACK quantization along the seq
         axis: q_l = fp8(x_l + e_{l-1}), e_l = (x_l + e_{l-1}) - q_l. The
         prefix-sum errors telescope: sum q_l = sum x_l - e_{last}, so the
         pooled error is ~one quantization step / eff (norm rel err ~3e-3,
         ~6x inside the 2e-2 gate) while raw fp8 would be 3.6e-2.
  - Device reduction is a 0/1-mask matmul in fp8 DoubleRow mode: each block
    holds 256 rows as [128 part, Ko=2, 2048]; mask[p, ko, slot] in {0,1} fp8.
    psum[16, 512q] += mask_j[128,2,16].T @ block_j[128,2,512q] contracts over
    256 (p,ko) cells at 2 fp8/cell/cycle. No DVE pre-add, no pairing
    semantics - any row can sit in any cell. Division by eff happens on HOST
    after gather (weights stay exactly 1.0; no mask-precision loss).
  - DMA layout: host writes each DMA group's bytes PARTITION-MAJOR so every
    partition's load is one contiguous descriptor (gl*4 KiB). Groups
    alternate the two HWDGE rings (SP/ACT); leading single-block groups get
    the first matmul started early; trailing singles shorten the tail. The
    mask rides the GpSimd SWDGE queue, off both HWDGE rings.
  - Tail: PSUM->SBUF copy on DVE (ACT reading PSUM crashes the exec unit),
    one out-DMA per 512-col bank, host scatters rows back to batch order.
"""

import os

import ml_dtypes
import numpy as np

import concourse.bacc as bacc
import concourse.mybir as mybir
from concourse.tile import TileContext
from concourse.bass_utils import run_bass_kernel_spmd

B, L, D = 128, 1024, 2048
NCORES = 8
SLOTS = B // NCORES  # 16
P = 128              # partitions
KO = 2               # DoubleRow depth (2 fp8 rows per PE cell)
RPB = P * KO         # 256 rows per block
NTILE = 512          # matmul moving free dim (one PSUM bank of fp32)

USE_DR = os.environ.get("USE_DR", "1") == "1"
GROUP = int(os.environ.get("DMA_GROUP", "4"))        # blocks per DMA group
FIRST_SINGLES = int(os.environ.get("FIRST_SINGLES", "2"))
TILE_BUFS = int(os.environ.get("TILE_BUFS", "6"))

F8 = ml_dtypes.float8_e4m3

LAST_RESULTS = None  # BassKernelResults of the most recent device run


def _plan(eff):
    """Snake-assign sorted batches to cores; return (cores, nblocks)."""
    order = np.argsort(-eff, kind="stable")
    cores = [[] for _ in range(NCORES)]
    for i, idx in enumerate(order):
        blk, pos = divmod(i, NCORES)
        c = pos if blk % 2 == 0 else NCORES - 1 - pos
        cores[c].append(int(idx))
    max_rows = max(sum(int(eff[b]) for b in perm) for perm in cores)
    nblk = -(-max_rows // RPB)
    return cores, nblk


def _groups(nblk):
    """DMA group sizes: FIRST_SINGLES single blocks, GROUP-block runs, then
    single blocks for the final <=GROUP blocks (shortens the matmul trail
    that runs after the last HBM byte lands)."""
    out, j = [], 0
    while j < nblk:
        if len(out) < FIRST_SINGLES or nblk - j <= GROUP:
            gl = 1
        else:
            gl = min(GROUP, nblk - j)
        out.append((j, gl))
        j += gl
    return out


_PROGRAM_CACHE = {}


def _build_program(nblk):
    # Bacc (not raw Bass): its compile pass splits multi-sem waits and moves
    # matmul waits onto ldweights — walrus allows only 1 wait per instruction.
    nc = bacc.Bacc(None, target_bir_lowering=False)
    f8 = mybir.dt.float8e4
    f32 = mybir.dt.float32
    packed = nc.dram_tensor("packed", [nblk * KO * P * D], f8, kind="ExternalInput")
    maskt = nc.dram_tensor("maskt", [P, nblk * KO * SLOTS], f8, kind="ExternalInput")
    out = nc.dram_tensor("out", [SLOTS, D], f32, kind="ExternalOutput")

    with TileContext(nc) as tc:
        with (
            tc.tile_pool(name="mask", bufs=1) as mpool,
            tc.tile_pool(name="tiles", bufs=TILE_BUFS) as tpool,
            tc.tile_pool(name="psum", bufs=1, space="PSUM") as ppool,
            tc.tile_pool(name="outs", bufs=1) as opool,
        ):
            mask_tile = mpool.tile([P, nblk * KO * SLOTS], f8)
            # One PSUM tile per 512-col bank so each bank's tail copy only
            # depends on ITS accumulation group's stop, not the whole psum.
            psums = [
                ppool.tile([SLOTS, NTILE], f32, name=f"ps{q}", tag=f"ps{q}")
                for q in range(D // NTILE)
            ]

            # Mask via SWDGE (GpSimd) so both HWDGE rings lead with data.
            nc.gpsimd.dma_start(out=mask_tile[:], in_=maskt[:])
            dma_engines = [nc.sync, nc.scalar]
            for n_dma, (j0, gl) in enumerate(_groups(nblk)):
                tile = tpool.tile([P, gl * KO * D], f8, name=f"t{gl}", tag="t")
                off = j0 * KO * P * D
                cnt = gl * KO * P * D
                src = packed[off : off + cnt].rearrange("(p x) -> p x", p=P)
                dma_engines[n_dma % 2].dma_start(out=tile[:], in_=src)
                for g in range(gl):
                    jj = j0 + g
                    if USE_DR:
                        blk = tile[:, g * KO * D : (g + 1) * KO * D].rearrange(
                            "p (ko x) -> p ko x", ko=KO
                        )
                        msk = mask_tile[
                            :, jj * KO * SLOTS : (jj + 1) * KO * SLOTS
                        ].rearrange("p (ko m) -> p ko m", ko=KO)
                        for q in range(D // NTILE):
                            nc.tensor.matmul(
                                psums[q][:, :],
                                msk,
                                blk[:, :, q * NTILE : (q + 1) * NTILE],
                                start=(jj == 0),
                                stop=(jj == nblk - 1),
                                perf_mode=mybir.MatmulPerfMode.DoubleRow,
                            )
                    else:
                        for k in range(KO):
                            c0 = (g * KO + k) * D
                            m0 = (jj * KO + k) * SLOTS
                            for q in range(D // NTILE):
                                nc.tensor.matmul(
                                    psums[q][:, :],
                                    mask_tile[:, m0 : m0 + SLOTS],
                                    tile[:, c0 + q * NTILE : c0 + (q + 1) * NTILE],
                                    start=(jj == 0 and k == 0),
                                    stop=(jj == nblk - 1 and k == KO - 1),
                                )

            # Tail: PSUM->SBUF via DVE (ACT reading PSUM crashes the exec
            # unit on this part). Bank q's copy chases its own group stop,
            # and each piece's out-DMA dispatch hides under the next copy.
            out_t = opool.tile([SLOTS, D], f32)
            for q in range(D // NTILE):
                nc.vector.tensor_copy(
                    out=out_t[:, q * NTILE : (q + 1) * NTILE], in_=psums[q][:, :]
                )
                dma_engines[q % 2].dma_start(
                    out=out[:, q * NTILE : (q + 1) * NTILE],
                    in_=out_t[:, q * NTILE : (q + 1) * NTILE],
                )
    nc.finalize()
    return nc


def _ef_quant(x):
    """Error-feedback (noise-shaping) fp8e4m3 quantization along axis 1.

    Returns uint8 view [B, L, D]. Prefix sums of the returned values match
    the fp32 prefix sums to within one final quantization step."""
    Bn, Ln, Dn = x.shape
    q = np.empty((Bn, Ln, Dn), dtype=np.uint8)
    e = np.zeros((Bn, Dn), dtype=np.float32)
    for l in range(Ln):
        t = x[:, l, :] + e
        ql = t.astype(F8)
        q[:, l, :] = ql.view(np.uint8)
        e = t - ql.astype(np.float32)
    return q


def kernel(features, lengths):
    global LAST_RESULTS
    features = np.ascontiguousarray(features, dtype=np.float32)
    lengths = np.ascontiguousarray(lengths, dtype=np.int32)
    eff = np.where(lengths > 0, lengths, L).astype(np.int64)

    cores, nblk = _plan(eff)
    key = (nblk, USE_DR, GROUP, FIRST_SINGLES, TILE_BUFS)
    if key not in _PROGRAM_CACHE:
        _PROGRAM_CACHE[key] = _build_program(nblk)
    nc = _PROGRAM_CACHE[key]
    groups = _groups(nblk)

    qrows = _ef_quant(features)  # [B, L, D] uint8 (fp8 bits)

    in_maps = []
    for c in range(NCORES):
        perm = cores[c]
        nrows = nblk * RPB
        rows = np.zeros((nrows, D), dtype=np.uint8)  # pad rows = fp8 +0.0
        slot = np.full(nrows, -1, dtype=np.int64)
        o = 0
        for s, b in enumerate(perm):
            e = int(eff[b])
            rows[o : o + e] = qrows[b, :e]
            slot[o : o + e] = s
            o += e
        # row r lives at block j=r//256, ko=(r%256)//128, p=r%128
        rows4 = rows.reshape(nblk, KO, P, D)
        flat = np.empty(nblk * KO * P * D, dtype=np.uint8)
        for j0, gl in groups:
            off = j0 * KO * P * D
            cnt = gl * KO * P * D
            seg = flat[off : off + cnt].reshape(P, gl, KO, D)
            seg[:] = rows4[j0 : j0 + gl].transpose(2, 0, 1, 3)
        maskf = np.zeros((nrows, SLOTS), dtype=np.float32)
        valid = slot >= 0
        maskf[np.arange(nrows)[valid], slot[valid]] = 1.0
        maskt = np.ascontiguousarray(
            maskf.astype(F8)
            .reshape(nblk, KO, P, SLOTS)
            .transpose(2, 0, 1, 3)
            .reshape(P, nblk * KO * SLOTS)
        )
        in_maps.append(
            {"packed": flat.view(F8), "maskt": maskt}
        )

    trace = os.environ.get("KERNEL_TRACE", "0") == "1"
    LAST_RESULTS = run_bass_kernel_spmd(
        nc,
        in_maps,
        core_ids=list(range(NCORES)),
        trace=trace,
        trace_cores=[0] if trace else None,
    )

    out = np.empty((B, D), dtype=np.float32)
    for c in range(NCORES):
        bidx = np.asarray(cores[c])
        out[bidx] = LAST_RESULTS.results[c]["out"] / eff[bidx, None]
    return out
